# revision 51
# baseline (speedup 1.0000x reference)
"""Trainium2 Bass kernel for KnowledgeAugmentedFusion.

  v = visual @ Wv.T + bv                      [B, D]
  t = text @ Wt.T + bt                        [B, D]
  k = knowledge @ Wk.T + bk                   [B, D]
  s = einsum('bj,ijl,bl->bi', t, W3, k)       [B, D]   (W3: [D, D, D])
  out = LayerNorm((v * s) @ Wo.T + bo)        [B, D]

Sharding: W3 along output-channel axis i across 8 cores (64 rows each);
t/k projections are tensor-parallel over their output dim (AllGather of the
tiny [64, 16] activation slices); v and the output layer are redundant.

Dataflow (per core, per output channel i):
  u[l, b]   = sum_j W3[i, j, l] * t[b, j]     16 matmuls with W3 as the
                                              *stationary* operand (Ldweights)
                                              and tT [128j, 16b] moving
  prod[l,b] = u[l, b] * kT[l, b]              one DVE tensor_tensor
  s[b, i]   = sum_l prod[l, b]                4 ones-matmuls accumulating into
                                              a persistent PSUM column S[:, i]
W3 streams as the stationary operand so the PE only ever streams 16-wide
moving operands; the kernel is DMA-bound on W3's HBM read.

W3 is cast to fp8 E3M4 on the host (4 mantissa bits keep end-to-end rel err
~1.3e-2, under the 2e-2 gate, where e4m3 would fail at 2.6e-2).  W3 is
pre-scaled by 256; the 1/256 is folded into Wk/bk on the host.

Epilogue: Wo's columns and bo are mean-centered on the host (LayerNorm's
mean subtraction is linear, so centering the weights makes the projection
output exactly zero-mean), removing the runtime mean reduction.  The output
projection runs transposed (Wo stationary, fusedT moving -> 16-wide matmuls),
variance comes from a Square-accumulate plus tiled-identity matmul reduce,
and the rstd broadcast is another tiny identity matmul.

All small constants and the sharded projection weights live in one packed
[128, CPK] bf16 tensor (single large-descriptor DMA).  W3 moves in 2-row
(512 KiB) chunks on the ScalarE DMA queue.
"""

import sys

if "/opt/trn_rl_repo" not in sys.path:
    sys.path.insert(0, "/opt/trn_rl_repo")

import numpy as np
import ml_dtypes

B = 16
VD, TD, KD, D = 2048, 768, 1024, 512
NCORES = 8
DSH = D // NCORES  # 64 output channels per core
LN_EPS = 1e-5
IC = 2             # i-rows per W3 DMA chunk
NCHUNK = DSH // IC

BF16 = ml_dtypes.bfloat16
F8E3 = ml_dtypes.float8_e3m4
W3_SCALE = 256.0

# constpack column layout (bf16, [128, CPK])
CP_TEXT = 0            # 6ct x 16b           textT (full)
CP_KNOW = 96           # 8ct x 16b           knowT (full)
CP_VIS = 224           # 16ct x 16b          visT (full)
CP_WVS = 480           # 16ct x 64d          Wv.T shard
CP_WTS = 1504          # 6ct x 64j           Wt.T shard
CP_WKS = 1888          # 8ct x 64l           Wk.T shard (pre/256)
CP_F32 = 2400          # fp32-upconvert region: [btk 2][IDT 64]
CP_BTK = 2400          # 2   (rows<64: bt shard, bk shard/256)
CP_IDT = 2402          # 64  (rows<16: tile(eye16,(1,4)))
NF32 = 66
CP_ID16 = 2466         # 16  (rows<16: eye16)
CP_ID128 = 2482        # 128 (eye128)
CP_ONE = 2610          # 1   (all ones)
CP_OR16 = 2611         # 16  (row0 ones)
CP_BV = 2627           # 64  (rows<16: bv shard rep)
CP_G2 = 2691           # 128 (rows<64: gamma[(r//16)*128 + p])
CP_BE2 = 2819          # 128 (rows<64: beta[...])
CPK = 2947

_CACHE = {}
LAST = {}


def _build_module(w3_bufs=8):
    import os
    n_chunks = int(os.environ.get("K_NC", str(NCHUNK)))
    use_cc = os.environ.get("K_CC", "1") == "1"
    body = os.environ.get("K_BODY", "full")
    from concourse import bacc, tile, mybir

    fp32 = mybir.dt.float32
    bf16 = mybir.dt.bfloat16
    f8e3 = mybir.dt.float8e3
    OP = mybir.AluOpType
    ACT = mybir.ActivationFunctionType

    nc = bacc.Bacc("TRN2", target_bir_lowering=False, debug=False,
                   num_devices=NCORES)

    # ---- DRAM I/O ----------------------------------------------------
    w3s = nc.dram_tensor("w3s", [DSH, D, D], f8e3, kind="ExternalInput")
    cpack = nc.dram_tensor("cpack", [128, CPK], bf16, kind="ExternalInput")
    woT = nc.dram_tensor("woT", [D, D], bf16, kind="ExternalInput")
    bocr = nc.dram_tensor("bocr", [1, D], bf16, kind="ExternalInput")
    out = nc.dram_tensor("out", [B, D], fp32, kind="ExternalOutput")
    use_dbg = os.environ.get("K_DBG", "0") == "1"
    if use_dbg:
        dbg_t = nc.dram_tensor("dbg_t", [128, 4 * B], fp32,
                               kind="ExternalOutput")
        dbg_k = nc.dram_tensor("dbg_k", [128, 4 * B], fp32,
                               kind="ExternalOutput")
        dbg_s = nc.dram_tensor("dbg_s", [B, DSH], fp32,
                               kind="ExternalOutput")
        dbg_f = nc.dram_tensor("dbg_f", [128, 4 * B], fp32,
                               kind="ExternalOutput")
        dbg_x = nc.dram_tensor("dbg_x", [128, 4 * B], fp32,
                               kind="ExternalOutput")
        dbg_v = nc.dram_tensor("dbg_v", [B, DSH], fp32,
                               kind="ExternalOutput")
        dbg_fu = nc.dram_tensor("dbg_fu", [B, DSH], fp32,
                                kind="ExternalOutput")
        dbg_fl = nc.dram_tensor("dbg_fl", [DSH, B], fp32,
                                kind="ExternalOutput")

    with tile.TileContext(nc) as tc:
        with tc.tile_pool(name="const", bufs=1) as constp, \
             tc.tile_pool(name="w3p", bufs=w3_bufs) as w3p, \
             tc.tile_pool(name="scr", bufs=3) as scrp, \
             tc.tile_pool(name="pp", bufs=3, space="PSUM") as pp, \
             tc.tile_pool(name="pi", bufs=3, space="PSUM") as pip, \
             tc.tile_pool(name="ps_s", bufs=1, space="PSUM") as pss, \
             tc.tile_pool(name="dram", bufs=1, space="DRAM") as dramp:

            # ---- ACT table preload (Square/Sqrt used in the LN tail) -
            jz = constp.tile([1, 1], fp32)
            nc.vector.memset(jz[:], 0.0)
            j1 = constp.tile([1, 1], fp32)
            j2 = constp.tile([1, 1], fp32)
            nc.scalar.activation(out=j1[:], in_=jz[:], func=ACT.Square,
                                 bias=jz[:])
            nc.scalar.activation(out=j2[:], in_=jz[:], func=ACT.Sqrt,
                                 bias=jz[:])

            # ---- SP queue: packed constants, then Wo -----------------
            cp_sb = constp.tile([128, CPK], bf16)
            nc.sync.dma_start(out=cp_sb[:], in_=cpack.ap())
            woT_sb = constp.tile([128, 4 * D], bf16)
            nc.sync.dma_start(out=woT_sb[:].rearrange("p (c d) -> p c d", c=4),
                              in_=woT.ap().rearrange("(c p) d -> p c d", p=128))
            bocr_sb = constp.tile([1, D], bf16)
            nc.sync.dma_start(out=bocr_sb[:], in_=bocr.ap())

            # fp32 upconvert of scalar/identity constants
            f32c = constp.tile([128, NF32], fp32)
            nc.vector.tensor_copy(f32c[:], cp_sb[:, CP_F32: CP_F32 + NF32])
            eps_t = constp.tile([DSH, 1], fp32)
            nc.vector.memset(eps_t[:], LN_EPS)

            # ---- t slice = (text @ Wt.T + bt)[:, shard] as [64j, 16b] -
            tT_ps = pp.tile([DSH, B], fp32, tag="pp")
            for ct in range(6):
                nc.tensor.matmul(
                    out=tT_ps[:],
                    lhsT=cp_sb[:, CP_WTS + DSH * ct: CP_WTS + DSH * ct + DSH],
                    rhs=cp_sb[:, CP_TEXT + B * ct: CP_TEXT + B * ct + B],
                    start=(ct == 0), stop=(ct == 5))
            tT_loc = constp.tile([DSH, B], bf16)
            nc.vector.tensor_scalar(out=tT_loc[:], in0=tT_ps[:],
                                    scalar1=f32c[0:DSH, 0:1], scalar2=None,
                                    op0=OP.add)

            # ---- k slice = (knowledge @ Wk.T + bk)/256 [64l, 16b] ----
            kT_ps = pp.tile([DSH, B], fp32, tag="pp")
            for ct in range(8):
                nc.tensor.matmul(
                    out=kT_ps[:],
                    lhsT=cp_sb[:, CP_WKS + DSH * ct: CP_WKS + DSH * ct + DSH],
                    rhs=cp_sb[:, CP_KNOW + B * ct: CP_KNOW + B * ct + B],
                    start=(ct == 0), stop=(ct == 7))
            kT_loc = constp.tile([DSH, B], bf16)
            nc.vector.tensor_scalar(out=kT_loc[:], in0=kT_ps[:],
                                    scalar1=f32c[0:DSH, 1:2], scalar2=None,
                                    op0=OP.add)

            # ---- AllGather t and k slices -> [128, (4g, 16b)] --------
            tT_sb = constp.tile([128, 4 * B], bf16)
            kT_sb = constp.tile([128, 4 * B], bf16)
            cc_t_in = dramp.tile([DSH, B], bf16)
            nc.sync.dma_start(out=cc_t_in[:], in_=tT_loc[:])
            cc_k_in = dramp.tile([DSH, B], bf16)
            nc.sync.dma_start(out=cc_k_in[:], in_=kT_loc[:])
            cc_t_out = dramp.tile([NCORES, DSH, B], bf16)
            cc_k_out = dramp.tile([NCORES, DSH, B], bf16)
            if use_cc:
                nc.gpsimd.collective_compute(
                    "AllGather", OP.bypass,
                    replica_groups=[list(range(NCORES))],
                    ins=[cc_t_in.opt()], outs=[cc_t_out.opt()])
                nc.gpsimd.collective_compute(
                    "AllGather", OP.bypass,
                    replica_groups=[list(range(NCORES))],
                    ins=[cc_k_in.opt()], outs=[cc_k_out.opt()])
                tc.strict_bb_all_engine_barrier()
            nc.sync.dma_start(
                out=tT_sb[:].rearrange("p (g b) -> p g b", g=4),
                in_=cc_t_out[:].rearrange("(g c2) j b -> (c2 j) g b", c2=2))
            nc.sync.dma_start(
                out=kT_sb[:].rearrange("p (g b) -> p g b", g=4),
                in_=cc_k_out[:].rearrange("(g c2) j b -> (c2 j) g b", c2=2))

            # ---- persistent PSUM accumulator for s ------------------
            S_ps = pss.tile([B, DSH], fp32)

            # ---- v slice = visual @ WvT[:, shard] + bv, [16b, 64i] ---
            ps_v = pp.tile([B, DSH], fp32, tag="pp")
            for ct in range(16):
                nc.tensor.matmul(
                    out=ps_v[:],
                    lhsT=cp_sb[:, CP_VIS + B * ct: CP_VIS + B * ct + B],
                    rhs=cp_sb[:, CP_WVS + DSH * ct: CP_WVS + DSH * ct + DSH],
                    start=(ct == 0), stop=(ct == 15))
            v_sb = constp.tile([B, DSH], fp32)
            nc.vector.tensor_tensor(out=v_sb[:], in0=ps_v[:],
                                    in1=cp_sb[0:B, CP_BV: CP_BV + DSH],
                                    op=OP.add)

            # ---- main loop: W3 chunks on the ACT DMA queue -----------
            # last two chunks are 1-row so the final chunk's DMA exposes
            # less latency on the critical path
            chunk_ranges = []
            i0 = 0
            for c in range(n_chunks):
                ni = IC if i0 + IC < DSH - 1 else 1
                if i0 >= DSH:
                    break
                chunk_ranges.append((i0, ni))
                i0 += ni
            while i0 < DSH:
                chunk_ranges.append((i0, 1))
                i0 += 1
            # red-matmuls for iteration i are emitted during iteration i+1
            # (software pipelining) so the PE never stalls waiting on the
            # DVE product of the chunk it just multiplied
            def emit_red(pend):
                prev_prod, prev_i = pend
                for lc in range(4):
                    nc.tensor.matmul(
                        out=S_ps[:, prev_i: prev_i + 1],
                        lhsT=prev_prod[:, B * lc: B * lc + B],
                        rhs=cp_sb[:, CP_ONE: CP_ONE + 1],
                        start=(lc == 0), stop=(lc == 3),
                        skip_group_check=True)

            pending = None
            for (c0, ni) in chunk_ranges:
                w3t = w3p.tile([128, IC * 4 * D], f8e3, tag="w3t")
                nc.scalar.dma_start(
                    out=w3t[:, : ni * 4 * D].rearrange(
                        "p (i jt l) -> p i jt l", i=ni, jt=4),
                    in_=w3s.ap()[c0: c0 + ni].rearrange(
                        "i (jt p) l -> p i jt l", p=128))
                if body == "dma":
                    continue
                for ii in range(ni):
                    i_g = c0 + ii
                    ps_u = pip.tile([128, 4 * B], fp32, tag="ps")
                    for lc in range(4):
                        for jc in range(4):
                            nc.tensor.matmul(
                                out=ps_u[:, B * lc: B * lc + B],
                                lhsT=w3t[:, (4 * ii + jc) * D + 128 * lc:
                                         (4 * ii + jc) * D + 128 * lc + 128],
                                rhs=tT_sb[:, B * jc: B * jc + B],
                                start=(jc == 0), stop=(jc == 3))
                    if body == "mm":
                        junk = scrp.tile([128, 4 * B], fp32, tag="junk")
                        nc.vector.tensor_copy(junk[:], ps_u[:])
                        continue
                    if pending is not None:
                        emit_red(pending)
                    prod = scrp.tile([128, 4 * B], bf16, tag="prod")
                    nc.vector.tensor_tensor(out=prod[:], in0=ps_u[:],
                                            in1=kT_sb[:], op=OP.mult)
                    pending = (prod, i_g)
            if pending is not None:
                emit_red(pending)

            # ---- fused = v * s  [16, 64] bf16; transpose; all-gather -
            fused_sb = constp.tile([B, DSH], bf16)
            nc.vector.tensor_tensor(out=fused_sb[:], in0=v_sb[:],
                                    in1=S_ps[:], op=OP.mult)
            ps_ft = pp.tile([DSH, B], bf16, tag="pp")
            nc.tensor.transpose(out=ps_ft[:], in_=fused_sb[:],
                                identity=cp_sb[0:B, CP_ID16: CP_ID16 + B])
            fusedT_loc = constp.tile([DSH, B], bf16)
            nc.vector.tensor_copy(fusedT_loc[:], ps_ft[:])

            # fusedT [512i, 16b] as [128, (4g, 16b)]; i = 128*g + p
            fusedT_sb = constp.tile([128, 4 * B], bf16)
            cc_f_in = dramp.tile([DSH, B], bf16)
            nc.scalar.dma_start(out=cc_f_in[:], in_=fusedT_loc[:])
            cc_f_out = dramp.tile([NCORES, DSH, B], bf16)
            if use_cc:
                nc.gpsimd.collective_compute(
                    "AllGather", OP.bypass,
                    replica_groups=[list(range(NCORES))],
                    ins=[cc_f_in.opt()], outs=[cc_f_out.opt()])
                tc.strict_bb_all_engine_barrier()
            nc.sync.dma_start(
                out=fusedT_sb[:].rearrange("p (g b) -> p g b", g=4),
                in_=cc_f_out[:].rearrange("(g c2) i b -> (c2 i) g b", c2=2))

            if use_dbg:
                d1 = scrp.tile([128, 4 * B], fp32, tag="d1")
                nc.vector.tensor_copy(d1[:], tT_sb[:])
                nc.sync.dma_start(out=dbg_t.ap(), in_=d1[:])
                d2 = scrp.tile([128, 4 * B], fp32, tag="d2")
                nc.vector.tensor_copy(d2[:], kT_sb[:])
                nc.sync.dma_start(out=dbg_k.ap(), in_=d2[:])
                d3 = scrp.tile([B, DSH], fp32, tag="d3")
                nc.vector.tensor_copy(d3[:], S_ps[:])
                nc.sync.dma_start(out=dbg_s.ap(), in_=d3[:])
                d4 = scrp.tile([128, 4 * B], fp32, tag="d4")
                nc.vector.tensor_copy(d4[:], fusedT_sb[:])
                nc.sync.dma_start(out=dbg_f.ap(), in_=d4[:])
                d6 = scrp.tile([B, DSH], fp32, tag="d6")
                nc.vector.tensor_copy(d6[:], v_sb[:])
                nc.sync.dma_start(out=dbg_v.ap(), in_=d6[:])
                d7 = scrp.tile([B, DSH], fp32, tag="d7")
                nc.vector.tensor_copy(d7[:], fused_sb[:])
                nc.sync.dma_start(out=dbg_fu.ap(), in_=d7[:])
                d8 = scrp.tile([DSH, B], fp32, tag="d8")
                nc.vector.tensor_copy(d8[:], fusedT_loc[:])
                nc.sync.dma_start(out=dbg_fl.ap(), in_=d8[:])

            # ---- epilogue: out = LN(fused @ Wo_c.T + bo_c) -----------
            # xcT[n, b] = sum_i Wo_c[n, i] fusedT[i, b] + bo_c[n]
            # (already exactly zero-mean over n by host centering)
            xcT_ps = pp.tile([128, 4 * B], fp32, tag="pp")
            for nc4 in range(4):
                for g in range(4):
                    nc.tensor.matmul(
                        out=xcT_ps[:, B * nc4: B * nc4 + B],
                        lhsT=woT_sb[:, D * g + 128 * nc4:
                                    D * g + 128 * nc4 + 128],
                        rhs=fusedT_sb[:, B * g: B * g + B],
                        start=(g == 0), stop=False)
                nc.tensor.matmul(
                    out=xcT_ps[:, B * nc4: B * nc4 + B],
                    lhsT=bocr_sb[0:1, 128 * nc4: 128 * nc4 + 128],
                    rhs=cp_sb[0:1, CP_OR16: CP_OR16 + B],
                    start=False, stop=True, skip_group_check=True)
            xcT_sb = constp.tile([128, 4 * B], bf16)
            nc.vector.tensor_copy(xcT_sb[:], xcT_ps[:])
            if use_dbg:
                d5 = scrp.tile([128, 4 * B], fp32, tag="d5")
                nc.vector.tensor_copy(d5[:], xcT_ps[:])
                nc.sync.dma_start(out=dbg_x.ap(), in_=d5[:])
            # transpose to xc2 [(4nc,16b), 128p] (PE) in parallel with the
            # variance reduction (DVE square + ones-matmul column sums)
            xc2_ps = pp.tile([DSH, 128], bf16, tag="pp")
            nc.tensor.transpose(out=xc2_ps[:], in_=xcT_sb[:],
                                identity=cp_sb[:, CP_ID128: CP_ID128 + 128])
            sq_sb = scrp.tile([128, 4 * B], bf16, tag="sq")
            nc.vector.tensor_tensor(out=sq_sb[:], in0=xcT_sb[:],
                                    in1=xcT_sb[:], op=OP.mult)
            var_ps = pp.tile([B, 1], fp32, tag="pp")
            for nc4 in range(4):
                nc.tensor.matmul(out=var_ps[:],
                                 lhsT=sq_sb[:, B * nc4: B * nc4 + B],
                                 rhs=cp_sb[:, CP_ONE: CP_ONE + 1],
                                 start=(nc4 == 0), stop=(nc4 == 3),
                                 skip_group_check=True)
            var_sb = constp.tile([B, 1], fp32)
            nc.vector.tensor_copy(var_sb[:], var_ps[:])
            # replicate var[b] -> [(nc,b), 1] then sqrt once
            rep_ps = pp.tile([DSH, 1], fp32, tag="pp")
            nc.tensor.matmul(out=rep_ps[:],
                             lhsT=f32c[0:B, 2: 2 + DSH],
                             rhs=var_sb[:], start=True, stop=True)
            std_sb = constp.tile([DSH, 1], fp32)
            nc.scalar.activation(out=std_sb[:], in_=rep_ps[:],
                                 func=ACT.Sqrt, bias=eps_t[:],
                                 scale=1.0 / D)
            rstd_sb = constp.tile([DSH, 1], fp32)
            nc.vector.reciprocal(out=rstd_sb[:], in_=std_sb[:])
            # y = (xc2 * rstd) * gamma ; out = y + beta
            y_sb = scrp.tile([DSH, 128], fp32, tag="y")
            nc.vector.scalar_tensor_tensor(
                out=y_sb[:], in0=xc2_ps[:], scalar=rstd_sb[:],
                in1=cp_sb[0:DSH, CP_G2: CP_G2 + 128],
                op0=OP.mult, op1=OP.mult)
            out_sb = scrp.tile([DSH, 128], fp32, tag="o")
            nc.vector.tensor_tensor(out=out_sb[:], in0=y_sb[:],
                                    in1=cp_sb[0:DSH, CP_BE2: CP_BE2 + 128],
                                    op=OP.add)
            # single DMA: SBUF side stays a plain [64,128] partition-major AP
            # (partition-split APs on the SBUF side mis-lower); the DRAM side
            # iterates (c, b, p) which matches the SBUF row order r = c*16+b
            nc.sync.dma_start(
                out=out.ap().rearrange("b (c p) -> c b p", c=4),
                in_=out_sb[:])

    nc.compile()
    return nc


def _prep_in_maps(inputs):
    f32 = np.float32

    W3 = np.asarray(inputs["W3"], dtype=f32)
    WvT = np.ascontiguousarray(np.asarray(inputs["Wv"], dtype=f32).T)
    WtT = np.asarray(inputs["Wt"], dtype=f32).T          # [TD, D]
    WkT = np.asarray(inputs["Wk"], dtype=f32).T / W3_SCALE
    bv = np.asarray(inputs["bv"], dtype=f32)
    bt = np.asarray(inputs["bt"], dtype=f32)
    bk = np.asarray(inputs["bk"], dtype=f32) / W3_SCALE
    Wo = np.asarray(inputs["Wo"], dtype=f32)
    bo = np.asarray(inputs["bo"], dtype=f32)
    gamma = np.asarray(inputs["gamma"], dtype=f32)
    beta = np.asarray(inputs["beta"], dtype=f32)
    textT = np.asarray(inputs["text_features"], dtype=f32).T     # [TD, B]
    knowT = np.asarray(inputs["knowledge_features"], dtype=f32).T  # [KD, B]
    visT = np.asarray(inputs["visual_features"], dtype=f32).T    # [VD, B]

    Wo_c = Wo - Wo.mean(axis=0, keepdims=True)
    bo_c = bo - bo.mean()
    woT_c = np.ascontiguousarray(Wo_c.T).astype(BF16)    # [D_i, D_n]

    def fold(x, n_chunk, width):
        # [n_chunk*128, width] -> [128, n_chunk*width]
        return x.reshape(n_chunk, 128, width).transpose(1, 0, 2).reshape(
            128, n_chunk * width)

    in_maps = []
    for m in range(NCORES):
        sl = slice(DSH * m, DSH * (m + 1))
        cp = np.zeros((128, CPK), dtype=f32)
        cp[:, CP_TEXT: CP_TEXT + 96] = fold(textT, 6, B)
        cp[:, CP_KNOW: CP_KNOW + 128] = fold(knowT, 8, B)
        cp[:, CP_VIS: CP_VIS + 256] = fold(visT, 16, B)
        cp[:, CP_WVS: CP_WVS + 1024] = fold(WvT[:, sl], 16, DSH)
        cp[:, CP_WTS: CP_WTS + 384] = fold(WtT[:, sl], 6, DSH)
        cp[:, CP_WKS: CP_WKS + 512] = fold(WkT[:, sl], 8, DSH)
        cp[:DSH, CP_BTK] = bt[sl]
        cp[:DSH, CP_BTK + 1] = bk[sl]
        cp[:B, CP_IDT: CP_IDT + DSH] = np.tile(np.eye(B, dtype=f32), (1, 4))
        cp[:B, CP_ID16: CP_ID16 + B] = np.eye(B, dtype=f32)
        cp[:, CP_ID128: CP_ID128 + 128] = np.eye(128, dtype=f32)
        cp[:, CP_ONE] = 1.0
        cp[0, CP_OR16: CP_OR16 + B] = 1.0
        cp[:B, CP_BV: CP_BV + DSH] = np.tile(bv[sl].reshape(1, DSH), (B, 1))
        rows = np.arange(DSH) // B            # nc index for row (nc*16+b)
        cp[:DSH, CP_G2: CP_G2 + 128] = gamma.reshape(4, 128)[rows]
        cp[:DSH, CP_BE2: CP_BE2 + 128] = beta.reshape(4, 128)[rows]
        per = {
            "w3s": (np.ascontiguousarray(W3[sl]) * W3_SCALE).astype(F8E3),
            "woT": woT_c,
            "bocr": bo_c.reshape(1, D).astype(BF16),
            "cpack": cp.astype(BF16),
        }
        in_maps.append(per)
    return in_maps


def kernel(**inputs):
    import os
    from concourse.bass_utils import run_bass_kernel_spmd

    if "nc" not in _CACHE:
        _CACHE["nc"] = _build_module()
    nc = _CACHE["nc"]

    in_maps = _prep_in_maps(inputs)
    trace = os.environ.get("KERNEL_TRACE", "0") == "1"
    res = run_bass_kernel_spmd(nc, in_maps, core_ids=list(range(NCORES)),
                               trace=trace)
    LAST["exec_time_ns"] = res.exec_time_ns
    LAST["results"] = res
    return np.asarray(res.results[0]["out"], dtype=np.float32)


# revision 60
# speedup vs baseline: 1.0039x; 1.0039x over previous
"""Trainium2 Bass kernel for KnowledgeAugmentedFusion.

  v = visual @ Wv.T + bv                      [B, D]
  t = text @ Wt.T + bt                        [B, D]
  k = knowledge @ Wk.T + bk                   [B, D]
  s = einsum('bj,ijl,bl->bi', t, W3, k)       [B, D]   (W3: [D, D, D])
  out = LayerNorm((v * s) @ Wo.T + bo)        [B, D]

Sharding: W3 along output-channel axis i across 8 cores (64 rows each);
t/k projections are tensor-parallel over their output dim (AllGather of the
tiny [64, 16] activation slices); v and the output layer are redundant.

Dataflow (per core, per output channel i):
  u[l, b]   = sum_j W3[i, j, l] * t[b, j]     16 matmuls with W3 as the
                                              *stationary* operand (Ldweights)
                                              and tT [128j, 16b] moving
  prod[l,b] = u[l, b] * kT[l, b]              one DVE tensor_tensor
  s[b, i]   = sum_l prod[l, b]                4 ones-matmuls accumulating into
                                              a persistent PSUM column S[:, i]
W3 streams as the stationary operand so the PE only ever streams 16-wide
moving operands; the kernel is DMA-bound on W3's HBM read.

W3 is cast to fp8 E3M4 on the host (4 mantissa bits keep end-to-end rel err
~1.3e-2, under the 2e-2 gate, where e4m3 would fail at 2.6e-2).  W3 is
pre-scaled by 256; the 1/256 is folded into Wk/bk on the host.

Epilogue: Wo's columns and bo are mean-centered on the host (LayerNorm's
mean subtraction is linear, so centering the weights makes the projection
output exactly zero-mean), removing the runtime mean reduction.  The output
projection runs transposed (Wo stationary, fusedT moving -> 16-wide matmuls),
variance comes from a Square-accumulate plus tiled-identity matmul reduce,
and the rstd broadcast is another tiny identity matmul.

All small constants and the sharded projection weights live in one packed
[128, CPK] bf16 tensor (single large-descriptor DMA).  W3 moves in 2-row
(512 KiB) chunks on the ScalarE DMA queue.
"""

import sys

if "/opt/trn_rl_repo" not in sys.path:
    sys.path.insert(0, "/opt/trn_rl_repo")

import numpy as np
import ml_dtypes

B = 16
VD, TD, KD, D = 2048, 768, 1024, 512
NCORES = 8
DSH = D // NCORES  # 64 output channels per core
LN_EPS = 1e-5
IC = 2             # i-rows per W3 DMA chunk
NCHUNK = DSH // IC

BF16 = ml_dtypes.bfloat16
F8E3 = ml_dtypes.float8_e3m4
W3_SCALE = 256.0

# constpack column layout (bf16, [128, CPK])
CP_TEXT = 0            # 6ct x 16b           textT (full)
CP_KNOW = 96           # 8ct x 16b           knowT (full)
CP_VIS = 224           # 16ct x 16b          visT (full)
CP_WVS = 480           # 16ct x 64d          Wv.T shard
CP_WTS = 1504          # 6ct x 64j           Wt.T shard
CP_WKS = 1888          # 8ct x 64l           Wk.T shard (pre/256)
CP_F32 = 2400          # fp32-upconvert region: [btk 2][IDT 64]
CP_BTK = 2400          # 2   (rows<64: bt shard, bk shard/256)
CP_IDT = 2402          # 64  (rows<16: tile(eye16,(1,4)))
NF32 = 66
CP_ID16 = 2466         # 16  (rows<16: eye16)
CP_ID128 = 2482        # 128 (eye128)
CP_ONE = 2610          # 1   (all ones)
CP_OR16 = 2611         # 16  (row0 ones)
CP_BV = 2627           # 64  (rows<16: bv shard rep)
CP_G2 = 2691           # 128 (rows<64: gamma[(r//16)*128 + p])
CP_BE2 = 2819          # 128 (rows<64: beta[...])
CPK = 2947

_CACHE = {}
LAST = {}


def _build_module(w3_bufs=8):
    import os
    n_chunks = int(os.environ.get("K_NC", str(NCHUNK)))
    use_cc = os.environ.get("K_CC", "1") == "1"
    body = os.environ.get("K_BODY", "full")
    from concourse import bacc, tile, mybir

    fp32 = mybir.dt.float32
    bf16 = mybir.dt.bfloat16
    f8e3 = mybir.dt.float8e3
    OP = mybir.AluOpType
    ACT = mybir.ActivationFunctionType

    nc = bacc.Bacc("TRN2", target_bir_lowering=False, debug=False,
                   num_devices=NCORES)

    # ---- DRAM I/O ----------------------------------------------------
    w3s = nc.dram_tensor("w3s", [DSH, D, D], f8e3, kind="ExternalInput")
    cpack = nc.dram_tensor("cpack", [128, CPK], bf16, kind="ExternalInput")
    woT = nc.dram_tensor("woT", [D, D], bf16, kind="ExternalInput")
    bocr = nc.dram_tensor("bocr", [1, D], bf16, kind="ExternalInput")
    out = nc.dram_tensor("out", [B, D], fp32, kind="ExternalOutput")
    use_dbg = os.environ.get("K_DBG", "0") == "1"
    if use_dbg:
        dbg_t = nc.dram_tensor("dbg_t", [128, 4 * B], fp32,
                               kind="ExternalOutput")
        dbg_k = nc.dram_tensor("dbg_k", [128, 4 * B], fp32,
                               kind="ExternalOutput")
        dbg_s = nc.dram_tensor("dbg_s", [B, DSH], fp32,
                               kind="ExternalOutput")
        dbg_f = nc.dram_tensor("dbg_f", [128, 4 * B], fp32,
                               kind="ExternalOutput")
        dbg_x = nc.dram_tensor("dbg_x", [128, 4 * B], fp32,
                               kind="ExternalOutput")
        dbg_v = nc.dram_tensor("dbg_v", [B, DSH], fp32,
                               kind="ExternalOutput")
        dbg_fu = nc.dram_tensor("dbg_fu", [B, DSH], fp32,
                                kind="ExternalOutput")
        dbg_fl = nc.dram_tensor("dbg_fl", [DSH, B], fp32,
                                kind="ExternalOutput")

    with tile.TileContext(nc) as tc:
        with tc.tile_pool(name="const", bufs=1) as constp, \
             tc.tile_pool(name="w3p", bufs=w3_bufs) as w3p, \
             tc.tile_pool(name="scr", bufs=3) as scrp, \
             tc.tile_pool(name="pp", bufs=3, space="PSUM") as pp, \
             tc.tile_pool(name="pi", bufs=3, space="PSUM") as pip, \
             tc.tile_pool(name="ps_s", bufs=1, space="PSUM") as pss, \
             tc.tile_pool(name="dram", bufs=1, space="DRAM") as dramp:

            # ---- ACT table preload (Square/Sqrt used in the LN tail) -
            jz = constp.tile([1, 1], fp32)
            nc.vector.memset(jz[:], 0.0)
            j1 = constp.tile([1, 1], fp32)
            j2 = constp.tile([1, 1], fp32)
            nc.scalar.activation(out=j1[:], in_=jz[:], func=ACT.Square,
                                 bias=jz[:])
            nc.scalar.activation(out=j2[:], in_=jz[:], func=ACT.Sqrt,
                                 bias=jz[:])

            # ---- SP queue: packed constants, then Wo -----------------
            cp_sb = constp.tile([128, CPK], bf16)
            nc.sync.dma_start(out=cp_sb[:], in_=cpack.ap())
            woT_sb = constp.tile([128, 4 * D], bf16)
            nc.sync.dma_start(out=woT_sb[:].rearrange("p (c d) -> p c d", c=4),
                              in_=woT.ap().rearrange("(c p) d -> p c d", p=128))
            bocr_sb = constp.tile([1, D], bf16)
            nc.sync.dma_start(out=bocr_sb[:], in_=bocr.ap())

            # fp32 upconvert of scalar/identity constants
            f32c = constp.tile([128, NF32], fp32)
            nc.vector.tensor_copy(f32c[:], cp_sb[:, CP_F32: CP_F32 + NF32])
            eps_t = constp.tile([DSH, 1], fp32)
            nc.vector.memset(eps_t[:], LN_EPS)

            # ---- t slice = (text @ Wt.T + bt)[:, shard] as [64j, 16b] -
            tT_ps = pp.tile([DSH, B], fp32, tag="pp")
            for ct in range(6):
                nc.tensor.matmul(
                    out=tT_ps[:],
                    lhsT=cp_sb[:, CP_WTS + DSH * ct: CP_WTS + DSH * ct + DSH],
                    rhs=cp_sb[:, CP_TEXT + B * ct: CP_TEXT + B * ct + B],
                    start=(ct == 0), stop=(ct == 5))
            tT_loc = constp.tile([DSH, B], bf16)
            nc.vector.tensor_scalar(out=tT_loc[:], in0=tT_ps[:],
                                    scalar1=f32c[0:DSH, 0:1], scalar2=None,
                                    op0=OP.add)

            # ---- k slice = (knowledge @ Wk.T + bk)/256 [64l, 16b] ----
            kT_ps = pp.tile([DSH, B], fp32, tag="pp")
            for ct in range(8):
                nc.tensor.matmul(
                    out=kT_ps[:],
                    lhsT=cp_sb[:, CP_WKS + DSH * ct: CP_WKS + DSH * ct + DSH],
                    rhs=cp_sb[:, CP_KNOW + B * ct: CP_KNOW + B * ct + B],
                    start=(ct == 0), stop=(ct == 7))
            kT_loc = constp.tile([DSH, B], bf16)
            nc.vector.tensor_scalar(out=kT_loc[:], in0=kT_ps[:],
                                    scalar1=f32c[0:DSH, 1:2], scalar2=None,
                                    op0=OP.add)

            # ---- AllGather t and k slices -> [128, (4g, 16b)] --------
            tT_sb = constp.tile([128, 4 * B], bf16)
            kT_sb = constp.tile([128, 4 * B], bf16)
            cc_t_in = dramp.tile([DSH, B], bf16)
            nc.sync.dma_start(out=cc_t_in[:], in_=tT_loc[:])
            cc_k_in = dramp.tile([DSH, B], bf16)
            nc.sync.dma_start(out=cc_k_in[:], in_=kT_loc[:])
            cc_t_out = dramp.tile([NCORES, DSH, B], bf16)
            cc_k_out = dramp.tile([NCORES, DSH, B], bf16)
            if use_cc:
                nc.gpsimd.collective_compute(
                    "AllGather", OP.bypass,
                    replica_groups=[list(range(NCORES))],
                    ins=[cc_t_in.opt()], outs=[cc_t_out.opt()])
                nc.gpsimd.collective_compute(
                    "AllGather", OP.bypass,
                    replica_groups=[list(range(NCORES))],
                    ins=[cc_k_in.opt()], outs=[cc_k_out.opt()])
                tc.strict_bb_all_engine_barrier()
            nc.sync.dma_start(
                out=tT_sb[:].rearrange("p (g b) -> p g b", g=4),
                in_=cc_t_out[:].rearrange("(g c2) j b -> (c2 j) g b", c2=2))
            nc.sync.dma_start(
                out=kT_sb[:].rearrange("p (g b) -> p g b", g=4),
                in_=cc_k_out[:].rearrange("(g c2) j b -> (c2 j) g b", c2=2))

            # ---- persistent PSUM accumulator for s ------------------
            S_ps = pss.tile([B, DSH], fp32)

            # ---- v slice = visual @ WvT[:, shard] + bv, [16b, 64i] ---
            ps_v = pp.tile([B, DSH], fp32, tag="pp")
            for ct in range(16):
                nc.tensor.matmul(
                    out=ps_v[:],
                    lhsT=cp_sb[:, CP_VIS + B * ct: CP_VIS + B * ct + B],
                    rhs=cp_sb[:, CP_WVS + DSH * ct: CP_WVS + DSH * ct + DSH],
                    start=(ct == 0), stop=(ct == 15))
            v_sb = constp.tile([B, DSH], fp32)
            nc.vector.tensor_tensor(out=v_sb[:], in0=ps_v[:],
                                    in1=cp_sb[0:B, CP_BV: CP_BV + DSH],
                                    op=OP.add)

            # ---- main loop: W3 chunks on the ACT DMA queue -----------
            # last two chunks are 1-row so the final chunk's DMA exposes
            # less latency on the critical path
            chunk_ranges = []
            i0 = 0
            for c in range(n_chunks):
                ni = IC if False else 1
                if i0 >= DSH:
                    break
                chunk_ranges.append((i0, ni))
                i0 += ni
            while i0 < DSH:
                chunk_ranges.append((i0, 1))
                i0 += 1
            # red-matmuls for iteration i are emitted during iteration i+1
            # (software pipelining) so the PE never stalls waiting on the
            # DVE product of the chunk it just multiplied
            def emit_red(pend):
                prev_prod, prev_i = pend
                for lc in range(4):
                    nc.tensor.matmul(
                        out=S_ps[:, prev_i: prev_i + 1],
                        lhsT=prev_prod[:, B * lc: B * lc + B],
                        rhs=cp_sb[:, CP_ONE: CP_ONE + 1],
                        start=(lc == 0), stop=(lc == 3),
                        skip_group_check=True)

            pending = None
            for (c0, ni) in chunk_ranges:
                w3t = w3p.tile([128, IC * 4 * D], f8e3, tag="w3t")
                nc.scalar.dma_start(
                    out=w3t[:, : ni * 4 * D].rearrange(
                        "p (i jt l) -> p i jt l", i=ni, jt=4),
                    in_=w3s.ap()[c0: c0 + ni].rearrange(
                        "i (jt p) l -> p i jt l", p=128))
                if body == "dma":
                    continue
                for ii in range(ni):
                    i_g = c0 + ii
                    ps_u = pip.tile([128, 4 * B], fp32, tag="ps")
                    for lc in range(4):
                        for jc in range(4):
                            nc.tensor.matmul(
                                out=ps_u[:, B * lc: B * lc + B],
                                lhsT=w3t[:, (4 * ii + jc) * D + 128 * lc:
                                         (4 * ii + jc) * D + 128 * lc + 128],
                                rhs=tT_sb[:, B * jc: B * jc + B],
                                start=(jc == 0), stop=(jc == 3))
                    if body == "mm":
                        junk = scrp.tile([128, 4 * B], fp32, tag="junk")
                        nc.vector.tensor_copy(junk[:], ps_u[:])
                        continue
                    if pending is not None:
                        emit_red(pending)
                    prod = scrp.tile([128, 4 * B], bf16, tag="prod")
                    nc.vector.tensor_tensor(out=prod[:], in0=ps_u[:],
                                            in1=kT_sb[:], op=OP.mult)
                    pending = (prod, i_g)
            if pending is not None:
                emit_red(pending)

            # ---- fused = v * s  [16, 64] bf16; transpose; all-gather -
            fused_sb = constp.tile([B, DSH], bf16)
            nc.vector.tensor_tensor(out=fused_sb[:], in0=v_sb[:],
                                    in1=S_ps[:], op=OP.mult)
            ps_ft = pp.tile([DSH, B], bf16, tag="pp")
            nc.tensor.transpose(out=ps_ft[:], in_=fused_sb[:],
                                identity=cp_sb[0:B, CP_ID16: CP_ID16 + B])
            fusedT_loc = constp.tile([DSH, B], bf16)
            nc.vector.tensor_copy(fusedT_loc[:], ps_ft[:])

            # fusedT [512i, 16b] as [128, (4g, 16b)]; i = 128*g + p
            fusedT_sb = constp.tile([128, 4 * B], bf16)
            cc_f_in = dramp.tile([DSH, B], bf16)
            nc.scalar.dma_start(out=cc_f_in[:], in_=fusedT_loc[:])
            cc_f_out = dramp.tile([NCORES, DSH, B], bf16)
            if use_cc:
                nc.gpsimd.collective_compute(
                    "AllGather", OP.bypass,
                    replica_groups=[list(range(NCORES))],
                    ins=[cc_f_in.opt()], outs=[cc_f_out.opt()])
                tc.strict_bb_all_engine_barrier()
            nc.sync.dma_start(
                out=fusedT_sb[:].rearrange("p (g b) -> p g b", g=4),
                in_=cc_f_out[:].rearrange("(g c2) i b -> (c2 i) g b", c2=2))

            if use_dbg:
                d1 = scrp.tile([128, 4 * B], fp32, tag="d1")
                nc.vector.tensor_copy(d1[:], tT_sb[:])
                nc.sync.dma_start(out=dbg_t.ap(), in_=d1[:])
                d2 = scrp.tile([128, 4 * B], fp32, tag="d2")
                nc.vector.tensor_copy(d2[:], kT_sb[:])
                nc.sync.dma_start(out=dbg_k.ap(), in_=d2[:])
                d3 = scrp.tile([B, DSH], fp32, tag="d3")
                nc.vector.tensor_copy(d3[:], S_ps[:])
                nc.sync.dma_start(out=dbg_s.ap(), in_=d3[:])
                d4 = scrp.tile([128, 4 * B], fp32, tag="d4")
                nc.vector.tensor_copy(d4[:], fusedT_sb[:])
                nc.sync.dma_start(out=dbg_f.ap(), in_=d4[:])
                d6 = scrp.tile([B, DSH], fp32, tag="d6")
                nc.vector.tensor_copy(d6[:], v_sb[:])
                nc.sync.dma_start(out=dbg_v.ap(), in_=d6[:])
                d7 = scrp.tile([B, DSH], fp32, tag="d7")
                nc.vector.tensor_copy(d7[:], fused_sb[:])
                nc.sync.dma_start(out=dbg_fu.ap(), in_=d7[:])
                d8 = scrp.tile([DSH, B], fp32, tag="d8")
                nc.vector.tensor_copy(d8[:], fusedT_loc[:])
                nc.sync.dma_start(out=dbg_fl.ap(), in_=d8[:])

            # ---- epilogue: out = LN(fused @ Wo_c.T + bo_c) -----------
            # xcT[n, b] = sum_i Wo_c[n, i] fusedT[i, b] + bo_c[n]
            # (already exactly zero-mean over n by host centering)
            xcT_ps = pp.tile([128, 4 * B], fp32, tag="pp")
            for nc4 in range(4):
                for g in range(4):
                    nc.tensor.matmul(
                        out=xcT_ps[:, B * nc4: B * nc4 + B],
                        lhsT=woT_sb[:, D * g + 128 * nc4:
                                    D * g + 128 * nc4 + 128],
                        rhs=fusedT_sb[:, B * g: B * g + B],
                        start=(g == 0), stop=False)
                nc.tensor.matmul(
                    out=xcT_ps[:, B * nc4: B * nc4 + B],
                    lhsT=bocr_sb[0:1, 128 * nc4: 128 * nc4 + 128],
                    rhs=cp_sb[0:1, CP_OR16: CP_OR16 + B],
                    start=False, stop=True, skip_group_check=True)
            xcT_sb = constp.tile([128, 4 * B], bf16)
            nc.vector.tensor_copy(xcT_sb[:], xcT_ps[:])
            if use_dbg:
                d5 = scrp.tile([128, 4 * B], fp32, tag="d5")
                nc.vector.tensor_copy(d5[:], xcT_ps[:])
                nc.sync.dma_start(out=dbg_x.ap(), in_=d5[:])
            # transpose to xc2 [(4nc,16b), 128p] (PE) in parallel with the
            # variance reduction (DVE square + ones-matmul column sums)
            xc2_ps = pp.tile([DSH, 128], bf16, tag="pp")
            nc.tensor.transpose(out=xc2_ps[:], in_=xcT_sb[:],
                                identity=cp_sb[:, CP_ID128: CP_ID128 + 128])
            sq_sb = scrp.tile([128, 4 * B], bf16, tag="sq")
            nc.vector.tensor_tensor(out=sq_sb[:], in0=xcT_sb[:],
                                    in1=xcT_sb[:], op=OP.mult)
            var_ps = pp.tile([B, 1], fp32, tag="pp")
            for nc4 in range(4):
                nc.tensor.matmul(out=var_ps[:],
                                 lhsT=sq_sb[:, B * nc4: B * nc4 + B],
                                 rhs=cp_sb[:, CP_ONE: CP_ONE + 1],
                                 start=(nc4 == 0), stop=(nc4 == 3),
                                 skip_group_check=True)
            var_sb = constp.tile([B, 1], fp32)
            nc.vector.tensor_copy(var_sb[:], var_ps[:])
            # replicate var[b] -> [(nc,b), 1] then sqrt once
            rep_ps = pp.tile([DSH, 1], fp32, tag="pp")
            nc.tensor.matmul(out=rep_ps[:],
                             lhsT=f32c[0:B, 2: 2 + DSH],
                             rhs=var_sb[:], start=True, stop=True)
            std_sb = constp.tile([DSH, 1], fp32)
            nc.scalar.activation(out=std_sb[:], in_=rep_ps[:],
                                 func=ACT.Sqrt, bias=eps_t[:],
                                 scale=1.0 / D)
            rstd_sb = constp.tile([DSH, 1], fp32)
            nc.vector.reciprocal(out=rstd_sb[:], in_=std_sb[:])
            # y = (xc2 * rstd) * gamma ; out = y + beta
            y_sb = scrp.tile([DSH, 128], fp32, tag="y")
            nc.vector.scalar_tensor_tensor(
                out=y_sb[:], in0=xc2_ps[:], scalar=rstd_sb[:],
                in1=cp_sb[0:DSH, CP_G2: CP_G2 + 128],
                op0=OP.mult, op1=OP.mult)
            out_sb = scrp.tile([DSH, 128], fp32, tag="o")
            nc.vector.tensor_tensor(out=out_sb[:], in0=y_sb[:],
                                    in1=cp_sb[0:DSH, CP_BE2: CP_BE2 + 128],
                                    op=OP.add)
            # single DMA: SBUF side stays a plain [64,128] partition-major AP
            # (partition-split APs on the SBUF side mis-lower); the DRAM side
            # iterates (c, b, p) which matches the SBUF row order r = c*16+b
            nc.sync.dma_start(
                out=out.ap().rearrange("b (c p) -> c b p", c=4),
                in_=out_sb[:])

    nc.compile()
    return nc


def _prep_in_maps(inputs):
    f32 = np.float32

    W3 = np.asarray(inputs["W3"], dtype=f32)
    WvT = np.ascontiguousarray(np.asarray(inputs["Wv"], dtype=f32).T)
    WtT = np.asarray(inputs["Wt"], dtype=f32).T          # [TD, D]
    WkT = np.asarray(inputs["Wk"], dtype=f32).T / W3_SCALE
    bv = np.asarray(inputs["bv"], dtype=f32)
    bt = np.asarray(inputs["bt"], dtype=f32)
    bk = np.asarray(inputs["bk"], dtype=f32) / W3_SCALE
    Wo = np.asarray(inputs["Wo"], dtype=f32)
    bo = np.asarray(inputs["bo"], dtype=f32)
    gamma = np.asarray(inputs["gamma"], dtype=f32)
    beta = np.asarray(inputs["beta"], dtype=f32)
    textT = np.asarray(inputs["text_features"], dtype=f32).T     # [TD, B]
    knowT = np.asarray(inputs["knowledge_features"], dtype=f32).T  # [KD, B]
    visT = np.asarray(inputs["visual_features"], dtype=f32).T    # [VD, B]

    Wo_c = Wo - Wo.mean(axis=0, keepdims=True)
    bo_c = bo - bo.mean()
    woT_c = np.ascontiguousarray(Wo_c.T).astype(BF16)    # [D_i, D_n]

    def fold(x, n_chunk, width):
        # [n_chunk*128, width] -> [128, n_chunk*width]
        return x.reshape(n_chunk, 128, width).transpose(1, 0, 2).reshape(
            128, n_chunk * width)

    in_maps = []
    for m in range(NCORES):
        sl = slice(DSH * m, DSH * (m + 1))
        cp = np.zeros((128, CPK), dtype=f32)
        cp[:, CP_TEXT: CP_TEXT + 96] = fold(textT, 6, B)
        cp[:, CP_KNOW: CP_KNOW + 128] = fold(knowT, 8, B)
        cp[:, CP_VIS: CP_VIS + 256] = fold(visT, 16, B)
        cp[:, CP_WVS: CP_WVS + 1024] = fold(WvT[:, sl], 16, DSH)
        cp[:, CP_WTS: CP_WTS + 384] = fold(WtT[:, sl], 6, DSH)
        cp[:, CP_WKS: CP_WKS + 512] = fold(WkT[:, sl], 8, DSH)
        cp[:DSH, CP_BTK] = bt[sl]
        cp[:DSH, CP_BTK + 1] = bk[sl]
        cp[:B, CP_IDT: CP_IDT + DSH] = np.tile(np.eye(B, dtype=f32), (1, 4))
        cp[:B, CP_ID16: CP_ID16 + B] = np.eye(B, dtype=f32)
        cp[:, CP_ID128: CP_ID128 + 128] = np.eye(128, dtype=f32)
        cp[:, CP_ONE] = 1.0
        cp[0, CP_OR16: CP_OR16 + B] = 1.0
        cp[:B, CP_BV: CP_BV + DSH] = np.tile(bv[sl].reshape(1, DSH), (B, 1))
        rows = np.arange(DSH) // B            # nc index for row (nc*16+b)
        cp[:DSH, CP_G2: CP_G2 + 128] = gamma.reshape(4, 128)[rows]
        cp[:DSH, CP_BE2: CP_BE2 + 128] = beta.reshape(4, 128)[rows]
        per = {
            "w3s": (np.ascontiguousarray(W3[sl]) * W3_SCALE).astype(F8E3),
            "woT": woT_c,
            "bocr": bo_c.reshape(1, D).astype(BF16),
            "cpack": cp.astype(BF16),
        }
        in_maps.append(per)
    return in_maps


def kernel(**inputs):
    import os
    from concourse.bass_utils import run_bass_kernel_spmd

    if "nc" not in _CACHE:
        _CACHE["nc"] = _build_module()
    nc = _CACHE["nc"]

    in_maps = _prep_in_maps(inputs)
    trace = os.environ.get("KERNEL_TRACE", "0") == "1"
    res = run_bass_kernel_spmd(nc, in_maps, core_ids=list(range(NCORES)),
                               trace=trace)
    LAST["exec_time_ns"] = res.exec_time_ns
    LAST["results"] = res
    return np.asarray(res.results[0]["out"], dtype=np.float32)
